# revision 39
# baseline (speedup 1.0000x reference)
"""Trainium2 Bass kernel for the sparse-attention decoder step (nn_Attention_35613868819045).

Math (per batch row b):
  pv[t,a]   = values[b,t,:] @ W_inputs                      (big matmul, fused below)
  conv[t,f] = SAME conv1d over stacked (old, cum) attention
  pa[t,a]   = conv @ W_loc          -> folded: pa^T = (lr @ W_loc)^T @ X  (X = shifted copies)
  pq[a]     = query[b] @ W_query
  score[t]  = tanh(pv + pq + pa) @ v_w          (+ v_b, which cancels in softmax)
  sn        = softmax(score);  u = 0.5*alpha + 0.5*shift(alpha) + 1e-8
  new_alpha = u*sn / sum(u*sn);  new_cum = cum + sn
  context   = new_alpha @ values[b]

Sharding: data-parallel, batch 64 -> 8 cores x 8. No collectives.
Device layout: feature-major (A on partitions, t free) so scores come out as
[1, t] rows on the PE; softmax/recursion run in [b, t] layout; context is a
bf16 PE matvec over natural-layout value tiles prefetched during phase 1.

Walrus allows only one sync-wait on a self-loading (fp32r) matmul, so all
weights arrive in ONE dma (host-packed blob), X arrives in one dma, and each
iteration starts with a PE nop that pre-absorbs the dma/ACT waits.
"""

import numpy as np
import ml_dtypes

import concourse.bass as bass
import concourse.bacc as bacc
import concourse.tile as tile
from concourse import mybir
from concourse.bass_utils import run_bass_kernel_spmd
from concourse.tile_rust import add_dep_helper

# problem sizes
B, T, DQ, DV, A, F, KW = 64, 1024, 1024, 512, 512, 32, 31
NCORES = 8
BS = B // NCORES          # 8 batches per core
TT = 512                  # t tile (one PSUM bank at fp32)
NTT = T // TT             # 2
PT = T + KW - 1           # padded attention length 1054
KX = 2 * KW               # 62 im2col rows
KXP = 72                  # X rows: 62 im2col + 2 zero + 8 one-hot(b) for the pq add

# weights split in three blobs: tiny one first (startup latency), then Wi, then Wq
OFF_VW = 0                        # [p, ac] = v_w[ac*128+p]             (4)
OFF_LR = OFF_VW + A // 128        # rows 0..31: [f, j] = lrp[j, f]      (64)
OFF_WL = OFF_LR + 64              # rows 0..31: W_loc                   (512)
OFF_EYE = OFF_WL + A              # rows 0..7: eye(8)                   (8)
NBS = OFF_EYE + BS
OFF_WI = 0                        # [p, dc*512 + a] = Wi[dc*128+p, a]   (2048)
NBW = DV // 128 * A
OFF_WQ = 0                        # [p, kc*512 + a] = Wq[kc*128+p, a]   (4096)
OFF_QT = OFF_WQ + DQ // 128 * A   # [p, kc*8 + b] = query[b, kc*128+p]  (64)
NBQ = OFF_QT + DQ // 128 * BS

import os
V_NO_LDWTOUCH = os.environ.get("KV_NO_LDWTOUCH") == "1"
V_NO_STRIDED = os.environ.get("KV_NO_STRIDED") == "1"
V_NO_PHASE3 = os.environ.get("KV_NO_PHASE3") == "1" or os.environ.get("KV_NO_PHASE2") == "1"
V_NO_PHASE2 = os.environ.get("KV_NO_PHASE2") == "1"
V_NO_PHASE1 = os.environ.get("KV_NO_PHASE1") == "1"
V_P2N = int(os.environ.get("KV_P2N", "99"))
if V_P2N < 99:
    V_NO_PHASE3 = True

f32 = mybir.dt.float32
f32r = mybir.dt.float32r
bf16 = mybir.dt.bfloat16
AF = mybir.ActivationFunctionType
ALU = mybir.AluOpType


def _order_after(mm, prev):
    """force `mm` to schedule after `prev` (no semaphore, same engine)."""
    if mm is not None and prev is not None:
        add_dep_helper(mm.ins, prev.ins, False, "order")


def _touch(nc, ap):
    """Standalone bf16 ldweights reading one element of `ap`: absorbs that
    tile's dma-lane wait as a natural data dep on the PE (TPB instructions
    carry at most ONE sync-wait). Writes nothing; fp32r matmuls self-load
    and phase-3 LDW/MM pairs are ordered after these, so the clobbered
    stationary weights are harmless."""
    if V_NO_LDWTOUCH:
        return None
    if ap.dtype == bf16:
        w = ap
    else:
        w = ap.bitcast(bf16)
    return nc.tensor.ldweights(w)


def _emit_score(nc, blobs_sb, ps_sc, sstage, th_list, b, tt):
    """stage score[b, tt*TT:+TT] = sum_ac v_w[ac].T @ tanh_tile[ac] at partition 32*(b//2)"""
    ps = ps_sc.tile([1, TT], f32, tag="sc")
    for ac in range(A // 128):
        nc.tensor.matmul(
            ps[:],
            blobs_sb[:, OFF_VW + ac : OFF_VW + ac + 1],
            th_list[ac][:],
            start=(ac == 0),
            stop=(ac == A // 128 - 1),
        )
    row = 32 * (b // 2)
    col = (b % 2) * T + tt * TT
    nc.vector.tensor_copy(sstage[row : row + 1, col : col + TT], ps[:])


def build_nc():
    nc = bacc.Bacc("TRN2", target_bir_lowering=False)

    # per-core inputs
    vT = nc.declare_dram_parameter("vT", [BS, DV, T], f32r, isOutput=False)
    vnat = nc.declare_dram_parameter("vnat", [BS, 128, T // 128, DV], bf16, isOutput=False)
    xh = nc.declare_dram_parameter("xh", [BS, KXP, T], f32r, isOutput=False)
    alpha = nc.declare_dram_parameter("alpha", [BS, T], f32, isOutput=False)
    cum = nc.declare_dram_parameter("cum", [BS, T], f32, isOutput=False)
    blob_s = nc.declare_dram_parameter("blob_s", [128, NBS], f32r, isOutput=False)
    blob_w = nc.declare_dram_parameter("blob_w", [128, NBW], f32r, isOutput=False)
    blob_q = nc.declare_dram_parameter("blob_q", [128, NBQ], f32r, isOutput=False)
    # outputs
    o_ctx = nc.declare_dram_parameter("o_ctx", [BS, DV], f32, isOutput=True)
    o_aw = nc.declare_dram_parameter("o_aw", [BS, T], f32, isOutput=True)
    o_cum = nc.declare_dram_parameter("o_cum", [BS, T], f32, isOutput=True)
    o_old = nc.declare_dram_parameter("o_old", [BS, T], f32, isOutput=True)
    o_na = nc.declare_dram_parameter("o_na", [BS, T], f32, isOutput=True)

    with tile.TileContext(nc) as tc:
        with (
            tc.tile_pool(name="cw", bufs=1) as cw,
            tc.tile_pool(name="xp", bufs=2) as xp,
            tc.tile_pool(name="vt", bufs=2) as vtp,
            tc.tile_pool(name="nat", bufs=1) as natp,
            tc.tile_pool(name="th", bufs=2) as thp,
            tc.tile_pool(name="ph2", bufs=1) as ph2,
            tc.tile_pool(name="ps_pv", bufs=4, space=bass.MemorySpace.PSUM) as ps_pv,
            tc.tile_pool(name="ps_sc", bufs=2, space=bass.MemorySpace.PSUM) as ps_sc,
            tc.tile_pool(name="ps_mi", bufs=2, space=bass.MemorySpace.PSUM) as ps_mi,
        ):
            # ---------------- phase 0: weights + small inputs ----------------
            blobs_sb = cw.tile([128, NBS], f32r)
            nc.gpsimd.dma_start(blobs_sb[:], blob_s[:])
            blobw_sb = cw.tile([128, NBW], f32r)
            nc.gpsimd.dma_start(blobw_sb[:], blob_w[:])
            blobq_sb = cw.tile([128, NBQ], f32r)
            nc.scalar.dma_start(blobq_sb[:], blob_q[:])
            al_sb = cw.tile([BS, T], f32)
            nc.gpsimd.dma_start(al_sb[:], alpha[:])
            cum_sb = cw.tile([BS, T], f32)
            nc.gpsimd.dma_start(cum_sb[:], cum[:])

            def wq(kc, ac):
                o = OFF_WQ + kc * A + ac * 128
                return blobq_sb[:, o : o + 128]

            def wi(dc, ac):
                o = OFF_WI + dc * A + ac * 128
                return blobw_sb[:, o : o + 128]

            def qt(kc):
                return blobq_sb[:, OFF_QT + kc * BS : OFF_QT + (kc + 1) * BS]

            lrT_sb = blobs_sb[0:F, OFF_LR : OFF_LR + 64]
            wloc_sb = blobs_sb[0:F, OFF_WL : OFF_WL + A]
            eye_sb = blobs_sb[0:BS, OFF_EYE : OFF_EYE + BS].bitcast(f32)

            # wmix rows 0..61 = lr @ Wloc (62..63 zero), rows 64..71 = pq[b]
            # so the K=72 "location" matmul over X also adds pq via X's one-hot rows.
            wmix = cw.tile([96, A], f32r)
            ps_wc = ps_mi.tile([64, A], f32, tag="mi")
            nc.tensor.matmul(ps_wc[:], lrT_sb, wloc_sb, start=True, stop=True)
            nc.scalar.copy(wmix[0:64, :], ps_wc[:])
            ps_pqn = ps_mi.tile([BS, A], f32, tag="mi")
            for kc in range(DQ // 128):
                nc.tensor.matmul(
                    ps_pqn[:], qt(kc), blobq_sb[:, OFF_WQ + kc * A : OFF_WQ + (kc + 1) * A],
                    start=(kc == 0), stop=(kc == DQ // 128 - 1),
                )
            nc.scalar.copy(wmix[64 : 64 + BS, :], ps_pqn[:])

            score_sb = cw.tile([BS, T], f32)
            # never-reused 1-elem ACT touch targets (no WAW -> single wait)
            ascr = cw.tile([1, 128], f32)
            # recursion numerator u = 0.5*(alpha + shift(alpha)) + 1e-8 (phase-0 precompute)
            sh_sb = cw.tile([BS, T], f32)
            nc.vector.memset(sh_sb[:, 0:1], 0.0)
            nc.vector.tensor_copy(sh_sb[:, 1:T], al_sb[:, 0 : T - 1])
            t1 = cw.tile([BS, T], f32)
            nc.vector.tensor_add(t1[:], al_sb[:], sh_sb[:])
            t2 = cw.tile([BS, T], f32)
            nc.vector.tensor_scalar(t2[:], t1[:], 0.5, 1e-8, ALU.mult, ALU.add)
            # score rows staged at 32-aligned partitions (DVE write restriction):
            # score(b) lives at partition 32*(b//2), col block (b%2)*T
            sstage = cw.tile([128, 2 * T], f32)

            # ---------------- phase 1: fused pv+pa+tanh+score ----------------
            nat_tiles = []
            pend = None          # (th_list, b, tt) score MMs, emitted one iter late
            last_mm = None       # last PV matmul of previous iteration
            first_mm_pending = None
            prev_act = None      # ACT-engine order chain
            n_touch = 0
            for b in range(0 if V_NO_PHASE1 else BS):
                # host-materialized im2col + one-hot(b) rows, one dma
                x_b = xp.tile([KXP, T], f32r, tag="x")
                nc.gpsimd.dma_start(x_b[:], xh[b])

                nat_b = natp.tile([128, T // 128, DV], bf16, tag=f"nat{b}")
                nc.gpsimd.dma_start(nat_b[:], vnat[b])
                nat_tiles.append(nat_b)

                vt_b = []
                for dc in range(DV // 128):
                    vt = vtp.tile([128, T], f32r, tag=f"vt{dc}")
                    nc.sync.dma_start(vt[:], vT[b, dc * 128 : (dc + 1) * 128, :])
                    vt_b.append(vt)

                tmm = None
                for tap in [v[0:1, 0:1] for v in vt_b] + [x_b[0:1, 0:1]]:
                    prev = tmm
                    tmm = _touch(nc, tap)
                    _order_after(tmm, prev if prev is not None else last_mm)
                first_mm_pending = tmm

                for tt in range(NTT):
                    ts = slice(tt * TT, (tt + 1) * TT)
                    th_list = []
                    for ac in range(A // 128):
                        ps = ps_pv.tile([128, TT], f32, tag="pv")
                        for dc in range(DV // 128):
                            mm = nc.tensor.matmul(
                                ps[:], wi(dc, ac), vt_b[dc][:, ts],
                                start=(dc == 0), stop=False,
                            )
                            if first_mm_pending is not None:
                                _order_after(mm, first_mm_pending)
                                first_mm_pending = None
                        last_mm = nc.tensor.matmul(
                            ps[:], wmix[0:KXP, ac * 128 : (ac + 1) * 128], x_b[:, ts],
                            start=False, stop=True,
                        )
                        th = thp.tile([128, TT], f32r, tag=f"th{ac}")
                        nc.scalar.activation(th[:], ps[:], AF.Tanh)
                        th_list.append(th)
                    if pend is not None:
                        _emit_score(nc, blobs_sb, ps_sc, sstage, *pend)
                    pend = (th_list, b, tt)
            if pend is not None:
                _emit_score(nc, blobs_sb, ps_sc, sstage, *pend)

            if V_NO_PHASE1:
                nc.vector.memset(score_sb[:], 0.0)
            elif V_NO_STRIDED:
                for bb_ in range(BS):
                    row = 32 * (bb_ // 2)
                    col = (bb_ % 2) * T
                    nc.gpsimd.dma_start(score_sb[bb_ : bb_ + 1, :], sstage[row : row + 1, col : col + T])
            else:
                nc.sync.dma_start(
                    score_sb[:], sstage[0:128:32, :].rearrange("p (k t) -> p k t", k=2)
                )

            # ---------------- phase 2: softmax + recursion ----------------
            if V_NO_PHASE2 or V_P2N < 99:
                nc.vector.memset(p_sb2 := ph2.tile([BS, T], f32, name="zz", tag="p_sb"), 0.0)
                for od in (o_na, o_aw, o_old, o_cum):
                    nc.sync.dma_start(od[:], p_sb2[:])
            run2 = not V_NO_PHASE2
            _p2 = [0]
            def g2():
                _p2[0] += 1
                return run2 and _p2[0] <= V_P2N
            p_sb = ph2.tile([BS, T], f32)
            s_sum = ph2.tile([BS, 1], f32)
            if g2():
                nc.scalar.activation(p_sb[:], score_sb[:], AF.Exp, accum_out=s_sum[:])
            rs = ph2.tile([BS, 1], f32)
            if g2():
                nc.vector.reciprocal(rs[:], s_sum[:])
            sn_sb = ph2.tile([BS, T], f32)
            if g2():
                nc.vector.tensor_scalar_mul(sn_sb[:], p_sb[:], rs[:])

            un = ph2.tile([BS, T], f32)
            s2 = ph2.tile([BS, 1], f32)
            if g2():
                nc.vector.tensor_mul(un[:], t2[:], sn_sb[:])
            if g2():
                nc.vector.tensor_reduce(s2[:], un[:], axis=mybir.AxisListType.X, op=ALU.add)
            r2 = ph2.tile([BS, 1], f32)
            if g2():
                nc.vector.reciprocal(r2[:], s2[:])
            na_sb = ph2.tile([BS, T], f32)
            if g2():
                nc.vector.tensor_scalar_mul(na_sb[:], un[:], r2[:])
            ncum_sb = ph2.tile([BS, T], f32)
            if g2():
                nc.vector.tensor_add(ncum_sb[:], cum_sb[:], sn_sb[:])

            if run2 and V_P2N >= 99:
                nc.gpsimd.dma_start(o_na[:], na_sb[:])
            if run2 and V_P2N >= 99:
                nc.gpsimd.dma_start(o_aw[:], na_sb[:])
            if run2 and V_P2N >= 99:
                nc.gpsimd.dma_start(o_old[:], na_sb[:])
            if run2 and V_P2N >= 99:
                nc.gpsimd.dma_start(o_cum[:], ncum_sb[:])

            # ---------------- phase 3: context = new_alpha @ values ----------------
            if V_NO_PHASE3 or V_NO_PHASE2:
                zc = ph2.tile([BS, DV], f32)
                nc.vector.memset(zc[:], 0.0)
                nc.gpsimd.dma_start(o_ctx[:], zc[:])
            tmm = last_mm
            for b in range(BS):
                if V_NO_PHASE3:
                    break
                prev = tmm
                tmm = _touch(nc, nat_tiles[b][0:1, 0, 0:2])
                _order_after(tmm, prev)
            nat_touch = tmm

            aT = []
            for tc8 in range(0 if V_NO_PHASE3 else T // 128):
                ps_t = ps_mi.tile([128, BS], f32, tag="mi")
                nc.tensor.transpose(ps_t[:], un[:, tc8 * 128 : (tc8 + 1) * 128], eye_sb)
                at = ph2.tile([128, BS], bf16, tag=f"aT{tc8}")
                nc.vector.tensor_copy(at[:], ps_t[:])
                aT.append(at)
            if not V_NO_PHASE3:
                ps_rt = ps_mi.tile([1, BS], f32, tag="mi")
                nc.tensor.transpose(ps_rt[:], r2[:], eye_sb)
                r2T = ph2.tile([1, BS], f32)
                nc.vector.tensor_copy(r2T[:], ps_rt[:])

            cstage = cw.tile([128, 2 * DV], f32)
            for bg in range(0 if V_NO_PHASE3 else BS // 4):
                ps_c = ps_sc.tile([128, DV], f32, tag="sc")
                for tc8 in range(T // 128):
                    for j in range(4):
                        b = bg * 4 + j
                        mm = nc.tensor.matmul(
                            ps_c[32 * j : 32 * j + 1, :],
                            aT[tc8][:, b : b + 1],
                            nat_tiles[b][:, tc8, :],
                            start=(tc8 == 0), stop=(tc8 == T // 128 - 1),
                            tile_position=(0, 32 * j),
                        )
                        if bg == 0 and tc8 == 0 and j == 0:
                            _order_after(mm, nat_touch)
                for j in range(4):
                    b = bg * 4 + j
                    row = 32 * (b // 2)
                    col = (b % 2) * DV
                    nc.scalar.activation(
                        cstage[row : row + 1, col : col + DV],
                        ps_c[32 * j : 32 * j + 1, :],
                        AF.Copy, bias=0.0, scale=r2T[0:1, b : b + 1],
                    )
            if V_NO_PHASE3:
                pass
            elif V_NO_STRIDED:
                for bb_ in range(BS):
                    row = 32 * (bb_ // 2)
                    col = (bb_ % 2) * DV
                    nc.gpsimd.dma_start(o_ctx[bb_ : bb_ + 1, :], cstage[row : row + 1, col : col + DV])
            else:
                nc.gpsimd.dma_start(
                    o_ctx[:], cstage[0:128:32, :].rearrange("p (k d) -> p k d", k=2)
                )

    nc.compile()
    return nc


def make_in_maps(query, values, attention_cum, attention_old, alpha,
                 W_query, W_inputs, v_w, v_b, loc_kernel, W_loc):
    query = np.asarray(query, np.float32)
    values = np.asarray(values, np.float32)
    attention_cum = np.asarray(attention_cum, np.float32)
    attention_old = np.asarray(attention_old, np.float32)
    alpha = np.asarray(alpha, np.float32)
    W_query = np.asarray(W_query, np.float32)
    W_inputs = np.asarray(W_inputs, np.float32)
    v_w = np.asarray(v_w, np.float32).reshape(A)
    loc_kernel = np.asarray(loc_kernel, np.float32)
    W_loc = np.asarray(W_loc, np.float32)
    # v_b shifts every score equally -> cancels in softmax; unused.
    lrp = np.zeros((64, F), np.float32)
    lrp[0:KW] = loc_kernel[:, 0, :]
    lrp[KW : 2 * KW] = loc_kernel[:, 1, :]

    blobs = np.zeros((128, NBS), np.float32)
    blobs[:, OFF_VW : OFF_VW + A // 128] = v_w.reshape(A // 128, 128).T
    blobs[0:F, OFF_LR : OFF_LR + 64] = lrp.T
    blobs[0:F, OFF_WL : OFF_WL + A] = W_loc
    blobs[0:BS, OFF_EYE : OFF_EYE + BS] = np.eye(BS, dtype=np.float32)
    blobw = np.zeros((128, NBW), np.float32)
    blobw[:, OFF_WI : OFF_WI + DV // 128 * A] = (
        W_inputs.reshape(DV // 128, 128, A).transpose(1, 0, 2).reshape(128, -1)
    )
    blobq_base = np.zeros((128, NBQ), np.float32)
    blobq_base[:, OFF_WQ : OFF_WQ + DQ // 128 * A] = (
        W_query.reshape(DQ // 128, 128, A).transpose(1, 0, 2).reshape(128, -1)
    )

    in_maps = []
    for i in range(NCORES):
        sl = slice(BS * i, BS * (i + 1))
        v = values[sl]
        # X[b]: rows k = old[b, t+k-15], rows 31+k = cum[b, t+k-15], 62-63 zero,
        # 64+j = (j == b), matching wmix rows so the K=72 matmul adds pa + pq.
        xh_w = np.zeros((BS, KXP, T), np.float32)
        pad_o = np.zeros((BS, PT), np.float32)
        pad_o[:, KW // 2 : KW // 2 + T] = attention_old[sl]
        pad_c = np.zeros((BS, PT), np.float32)
        pad_c[:, KW // 2 : KW // 2 + T] = attention_cum[sl]
        for k in range(KW):
            xh_w[:, k, :] = pad_o[:, k : k + T]
            xh_w[:, KW + k, :] = pad_c[:, k : k + T]
        for j in range(BS):
            xh_w[j, 64 + j, :] = 1.0
        blbq = blobq_base.copy()
        blbq[:, OFF_QT : OFF_QT + DQ // 128 * BS] = (
            query[sl].T.reshape(DQ // 128, 128, BS).transpose(1, 0, 2).reshape(128, -1)
        )
        m = {
            "vT": np.ascontiguousarray(v.transpose(0, 2, 1)),
            "vnat": np.ascontiguousarray(
                v.reshape(BS, T // 128, 128, DV).transpose(0, 2, 1, 3)
            ).astype(ml_dtypes.bfloat16),
            "xh": xh_w,
            "alpha": np.ascontiguousarray(alpha[sl]),
            "cum": np.ascontiguousarray(attention_cum[sl]),
            "blob_s": blobs,
            "blob_w": blobw,
            "blob_q": blbq,
        }
        in_maps.append(m)
    return in_maps


_NC_CACHE = {}


def kernel(**inputs):
    in_maps = make_in_maps(**inputs)
    if "nc" not in _NC_CACHE:
        _NC_CACHE["nc"] = build_nc()
    nc = _NC_CACHE["nc"]
    res = run_bass_kernel_spmd(nc, in_maps, core_ids=list(range(NCORES))).results
    context = np.concatenate([res[i]["o_ctx"] for i in range(NCORES)], axis=0)
    aw = np.concatenate([res[i]["o_aw"] for i in range(NCORES)], axis=0)
    ncum = np.concatenate([res[i]["o_cum"] for i in range(NCORES)], axis=0)
    nold = np.concatenate([res[i]["o_old"] for i in range(NCORES)], axis=0)
    na = np.concatenate([res[i]["o_na"] for i in range(NCORES)], axis=0)
    return (
        context.astype(np.float32),
        aw.astype(np.float32),
        ncum.astype(np.float32),
        nold.astype(np.float32),
        na.astype(np.float32),
    )
